# revision 5
# baseline (speedup 1.0000x reference)
"""Trainium2 Bass kernel for nn_ComplexBlockLinear.

Math: per block n (8 blocks of 128 features), out = x @ W[n] with complex
x = x_re + i*x_im, W = wr + i*wi:
    out_re = xr @ wr - xi @ wi
    out_im = xr @ wi + xi @ wr

Strategy:
  - Data parallel: core b handles batch element b (B=8, 8 cores).
  - Host: transpose x[b] to [H, S] (feature-major) so the contraction dim
    lands on SBUF partitions, and split fp32 into bf16 hi+lo (Dekker split).
    fp32 matmul on TRN2 PE costs 4 cycles/col; bf16 costs 1, so the 3-term
    bf16 product (hi*hi + hi*lo + lo*hi) runs at 3/4 the fp32 cost with
    ~1e-5 relative error. Accumulation is fp32 in PSUM.
  - Device: weights stationary ([128i, 128o] per block), stream token
    chunks of 512; 12 matmuls per (block, chunk) accumulate psum_re/psum_im;
    DVE evacuates PSUM->SBUF; 1-2MB HWDGE DMAs both directions.
  - Host: transpose outputs back and interleave re/im.
"""

import os

import numpy as np
import ml_dtypes

B, S, H = 8, 4096, 1024
NBLK, BS = 8, 128
NCORES = 8
TCHUNK = 512
NCHUNK = S // TCHUNK

BF16 = ml_dtypes.bfloat16

# stationary weight pack order along the free axis
WRH, WRL, WIH, WIL, WIHN, WILN = range(6)

_CACHE = {}


def _build_module(nblk, s, tchunk):
    import concourse.mybir as mybir
    from concourse import bacc
    from concourse.tile import TileContext

    dt = mybir.dt
    h = nblk * BS
    nchunk = s // tchunk

    nc = bacc.Bacc(
        "TRN2",
        target_bir_lowering=False,
        debug=False,
        enable_asserts=False,
        num_devices=NCORES,
    )

    xrh = nc.dram_tensor("xrh", [h, s], dt.bfloat16, kind="ExternalInput").ap()
    xrl = nc.dram_tensor("xrl", [h, s], dt.bfloat16, kind="ExternalInput").ap()
    xih = nc.dram_tensor("xih", [h, s], dt.bfloat16, kind="ExternalInput").ap()
    xil = nc.dram_tensor("xil", [h, s], dt.bfloat16, kind="ExternalInput").ap()
    wpack = nc.dram_tensor(
        "wpack", [nblk, BS, 6 * BS], dt.bfloat16, kind="ExternalInput"
    ).ap()
    out_re = nc.dram_tensor("out_re", [h, s], dt.float32, kind="ExternalOutput").ap()
    out_im = nc.dram_tensor("out_im", [h, s], dt.float32, kind="ExternalOutput").ap()

    # feature-blocked DRAM views: [h, s] -> [p, n, t]
    xrh_v = xrh.rearrange("(n p) t -> p n t", p=BS)
    xrl_v = xrl.rearrange("(n p) t -> p n t", p=BS)
    xih_v = xih.rearrange("(n p) t -> p n t", p=BS)
    xil_v = xil.rearrange("(n p) t -> p n t", p=BS)
    ore_v = out_re.rearrange("(n p) t -> p n t", p=BS)
    oim_v = out_im.rearrange("(n p) t -> p n t", p=BS)
    w_v = wpack.rearrange("n i s -> i n s")

    with TileContext(nc) as tc:
        with (
            tc.tile_pool(name="wpool", bufs=1) as wpool,
            tc.tile_pool(name="xpool", bufs=2) as xpool,
            tc.tile_pool(name="opool", bufs=2) as opool,
            tc.tile_pool(name="psum", bufs=3, space="PSUM") as psum_pool,
        ):
            wt = wpool.tile([BS, nblk * 6 * BS], dt.bfloat16)
            nc.sync.dma_start(
                out=wt.rearrange("p (n s) -> p n s", s=6 * BS), in_=w_v
            )

            def wsl(n, k):
                return wt[:, (n * 6 + k) * BS : (n * 6 + k + 1) * BS]

            for c in range(nchunk):
                tsl = slice(c * tchunk, (c + 1) * tchunk)

                xrh_t = xpool.tile([BS, nblk * tchunk], dt.bfloat16)
                xrl_t = xpool.tile([BS, nblk * tchunk], dt.bfloat16)
                xih_t = xpool.tile([BS, nblk * tchunk], dt.bfloat16)
                xil_t = xpool.tile([BS, nblk * tchunk], dt.bfloat16)
                for tile_, view in (
                    (xrh_t, xrh_v),
                    (xrl_t, xrl_v),
                    (xih_t, xih_v),
                    (xil_t, xil_v),
                ):
                    nc.sync.dma_start(
                        out=tile_.rearrange("p (n t) -> p n t", t=tchunk),
                        in_=view[:, :, tsl],
                    )

                osb_re = opool.tile([BS, nblk * tchunk], dt.float32)
                osb_im = opool.tile([BS, nblk * tchunk], dt.float32)

                for n in range(nblk):
                    bsl = slice(n * tchunk, (n + 1) * tchunk)
                    xr_h = xrh_t[:, bsl]
                    xr_l = xrl_t[:, bsl]
                    xi_h = xih_t[:, bsl]
                    xi_l = xil_t[:, bsl]

                    ps_re = psum_pool.tile([BS, tchunk], dt.float32)
                    ps_im = psum_pool.tile([BS, tchunk], dt.float32)
                    mm = nc.tensor.matmul
                    # grouped by stationary operand to minimize weight reloads
                    mm(ps_re, wsl(n, WRH), xr_h, start=True, stop=False)
                    mm(ps_re, wsl(n, WRH), xr_l, start=False, stop=False)
                    mm(ps_im, wsl(n, WRH), xi_h, start=True, stop=False)
                    mm(ps_im, wsl(n, WRH), xi_l, start=False, stop=False)
                    mm(ps_re, wsl(n, WRL), xr_h, start=False, stop=False)
                    mm(ps_im, wsl(n, WRL), xi_h, start=False, stop=False)
                    mm(ps_im, wsl(n, WIH), xr_h, start=False, stop=False)
                    mm(ps_im, wsl(n, WIH), xr_l, start=False, stop=False)
                    mm(ps_im, wsl(n, WIL), xr_h, start=False, stop=True)
                    mm(ps_re, wsl(n, WIHN), xi_h, start=False, stop=False)
                    mm(ps_re, wsl(n, WIHN), xi_l, start=False, stop=False)
                    mm(ps_re, wsl(n, WILN), xi_h, start=False, stop=True)

                    nc.vector.tensor_copy(osb_re[:, bsl], ps_re)
                    nc.vector.tensor_copy(osb_im[:, bsl], ps_im)

                nc.scalar.dma_start(
                    out=ore_v[:, :, tsl],
                    in_=osb_re.rearrange("p (n t) -> p n t", t=tchunk),
                )
                nc.scalar.dma_start(
                    out=oim_v[:, :, tsl],
                    in_=osb_im.rearrange("p (n t) -> p n t", t=tchunk),
                )

    nc.compile()
    return nc


def _get_module(nblk=NBLK, s=S, tchunk=TCHUNK):
    key = (nblk, s, tchunk)
    if key not in _CACHE:
        _CACHE[key] = _build_module(nblk, s, tchunk)
    return _CACHE[key]


def _split_bf16(x32):
    hi = x32.astype(BF16)
    lo = (x32 - hi.astype(np.float32)).astype(BF16)
    return np.ascontiguousarray(hi), np.ascontiguousarray(lo)


def _pack_weights(weight):
    wr = weight[..., 0].astype(np.float32)  # [n, i, o]
    wi = weight[..., 1].astype(np.float32)
    wrh = wr.astype(BF16)
    wrl = (wr - wrh.astype(np.float32)).astype(BF16)
    wih = wi.astype(BF16)
    wil = (wi - wih.astype(np.float32)).astype(BF16)
    return np.ascontiguousarray(
        np.concatenate([wrh, wrl, wih, wil, -wih, -wil], axis=2)
    )


def _setup_trace_shim():
    """Make trace=True work under axon in containers lacking antenv.axon_hooks.

    Registers a stand-in antenv.axon_hooks module whose hook drives NTFF
    capture via ctypes on libaxon_pjrt.so (mirrors trn_agent_boot), and
    disables the S3 artifact upload in bass_utils.
    """
    import contextlib
    import ctypes
    import sys
    import types

    try:
        from antenv.axon_hooks import get_axon_ntff_profile_hook  # noqa: F401

        return
    except ImportError:
        pass

    so_path = "/opt/axon/libaxon_pjrt.so"
    lib = ctypes.CDLL(so_path)
    if not hasattr(lib, "axon_start_nrt_profile"):
        return
    lib.axon_start_nrt_profile.argtypes = [
        ctypes.POINTER(ctypes.c_int64),
        ctypes.c_size_t,
    ]
    lib.axon_start_nrt_profile.restype = ctypes.c_int64
    lib.axon_stop_nrt_profile.argtypes = [ctypes.c_char_p]
    lib.axon_stop_nrt_profile.restype = ctypes.c_int64

    @contextlib.contextmanager
    def _hook(output_dir, device_ids):
        import jax

        jax.devices()
        if device_ids:
            ids = (ctypes.c_int64 * len(device_ids))(*device_ids)
            rc = lib.axon_start_nrt_profile(ids, len(device_ids))
        else:
            rc = lib.axon_start_nrt_profile(None, 0)
        if rc != 0:
            raise RuntimeError(f"axon_start_nrt_profile rc={rc}")
        try:
            yield
        finally:
            n = lib.axon_stop_nrt_profile(str(output_dir).encode())
            print(f"ntff profile: {n} file(s) written to {output_dir}")

    mod = types.ModuleType("antenv.axon_hooks")
    mod.get_axon_ntff_profile_hook = lambda: _hook
    mod.set_axon_ntff_profile_hook = lambda h: None
    sys.modules["antenv.axon_hooks"] = mod

    from concourse import bass_utils

    bass_utils.upload_artifacts = lambda tmpdir: tmpdir


def kernel(x_re, x_im, weight):
    from concourse import bass_utils

    trace = bool(int(os.environ.get("KERNEL_TRACE", "0")))
    if trace:
        _setup_trace_shim()

    nc = _get_module()
    wpack = _pack_weights(weight)

    in_maps = []
    for b in range(NCORES):
        d = {"wpack": wpack}
        d["xrh"], d["xrl"] = _split_bf16(x_re[b].T.astype(np.float32))
        d["xih"], d["xil"] = _split_bf16(x_im[b].T.astype(np.float32))
        in_maps.append(d)

    res = bass_utils.run_bass_kernel_spmd(
        nc,
        in_maps,
        core_ids=list(range(NCORES)),
        trace=trace,
    )
    kernel._last_results = res

    out = np.empty((B, S, H, 2), np.float32)
    for b in range(NCORES):
        out[b, :, :, 0] = res.results[b]["out_re"].T
        out[b, :, :, 1] = res.results[b]["out_im"].T
    return out


kernel._last_results = None


# revision 6
# speedup vs baseline: 1.0347x; 1.0347x over previous
"""Trainium2 Bass kernel for nn_ComplexBlockLinear.

Math: per block n (8 blocks of 128 features), out = x @ W[n] with complex
x = x_re + i*x_im, W = wr + i*wi:
    out_re = xr @ wr - xi @ wi
    out_im = xr @ wi + xi @ wr

Strategy:
  - Data parallel: core b handles batch element b (B=8, 8 cores).
  - Host: transpose x[b] to [H, S] (feature-major) so the contraction dim
    lands on SBUF partitions, and split fp32 into bf16 hi+lo (Dekker split).
    fp32 matmul on TRN2 PE costs 4 cycles/col; bf16 costs 1, so the 3-term
    bf16 product (hi*hi + hi*lo + lo*hi) runs at 3/4 the fp32 cost with
    ~1e-5 relative error. Accumulation is fp32 in PSUM.
  - Device: weights stationary ([128i, 128o] per block), stream token
    chunks of 512; 12 matmuls per (block, chunk) accumulate psum_re/psum_im;
    DVE evacuates PSUM->SBUF; 1-2MB HWDGE DMAs both directions.
  - Host: transpose outputs back and interleave re/im.
"""

import os

import numpy as np
import ml_dtypes

B, S, H = 8, 4096, 1024
NBLK, BS = 8, 128
NCORES = 8
TCHUNK = 512
NCHUNK = S // TCHUNK

BF16 = ml_dtypes.bfloat16

# stationary weight pack order along the free axis
WRH, WRL, WIH, WIL, WIHN, WILN = range(6)

_CACHE = {}


def _build_module(nblk, s, tchunk):
    import concourse.mybir as mybir
    from concourse import bacc
    from concourse.tile import TileContext

    dt = mybir.dt
    h = nblk * BS
    nchunk = s // tchunk

    nc = bacc.Bacc(
        "TRN2",
        target_bir_lowering=False,
        debug=False,
        enable_asserts=False,
        num_devices=NCORES,
    )

    xrh = nc.dram_tensor("xrh", [h, s], dt.bfloat16, kind="ExternalInput").ap()
    xrl = nc.dram_tensor("xrl", [h, s], dt.bfloat16, kind="ExternalInput").ap()
    xih = nc.dram_tensor("xih", [h, s], dt.bfloat16, kind="ExternalInput").ap()
    xil = nc.dram_tensor("xil", [h, s], dt.bfloat16, kind="ExternalInput").ap()
    wpack = nc.dram_tensor(
        "wpack", [nblk, BS, 6 * BS], dt.bfloat16, kind="ExternalInput"
    ).ap()
    out_re = nc.dram_tensor("out_re", [h, s], dt.float32, kind="ExternalOutput").ap()
    out_im = nc.dram_tensor("out_im", [h, s], dt.float32, kind="ExternalOutput").ap()

    # feature-blocked DRAM views: [h, s] -> [p, n, t]
    xrh_v = xrh.rearrange("(n p) t -> p n t", p=BS)
    xrl_v = xrl.rearrange("(n p) t -> p n t", p=BS)
    xih_v = xih.rearrange("(n p) t -> p n t", p=BS)
    xil_v = xil.rearrange("(n p) t -> p n t", p=BS)
    ore_v = out_re.rearrange("(n p) t -> p n t", p=BS)
    oim_v = out_im.rearrange("(n p) t -> p n t", p=BS)
    w_v = wpack.rearrange("n i s -> i n s")

    with TileContext(nc) as tc:
        with (
            tc.tile_pool(name="wpool", bufs=1) as wpool,
            tc.tile_pool(name="xpool", bufs=3) as xpool,
            tc.tile_pool(name="opool", bufs=4) as opool,
            tc.tile_pool(name="psum", bufs=4, space="PSUM") as psum_pool,
        ):
            wt = wpool.tile([BS, nblk * 6 * BS], dt.bfloat16)
            wt_v = wt.rearrange("p (n s) -> p n s", s=6 * BS)

            def load_w(n):
                nc.sync.dma_start(out=wt_v[:, n], in_=w_v[:, n])

            def wsl(n, k):
                return wt[:, (n * 6 + k) * BS : (n * 6 + k + 1) * BS]

            def load_x(c):
                tsl = slice(c * tchunk, (c + 1) * tchunk)
                tiles = []
                for nm, view in (
                    ("xrh_t", xrh_v),
                    ("xrl_t", xrl_v),
                    ("xih_t", xih_v),
                    ("xil_t", xil_v),
                ):
                    tile_ = xpool.tile([BS, nblk * tchunk], dt.bfloat16, name=nm)
                    nc.sync.dma_start(
                        out=tile_.rearrange("p (n t) -> p n t", t=tchunk),
                        in_=view[:, :, tsl],
                    )
                    tiles.append(tile_)
                return tiles

            mm = nc.tensor.matmul

            def mm_xr(n, ps_re, ps_im, xr_h, xr_l, first):
                # the 6 terms sourced from x_re (need only xrh/xrl slabs)
                bsl = slice(n * tchunk, (n + 1) * tchunk)
                a, b = xr_h[:, bsl], xr_l[:, bsl]
                mm(ps_re, wsl(n, WRH), a, start=first, stop=False)
                mm(ps_re, wsl(n, WRH), b, start=False, stop=False)
                mm(ps_im, wsl(n, WIH), a, start=first, stop=False)
                mm(ps_im, wsl(n, WIH), b, start=False, stop=False)
                mm(ps_re, wsl(n, WRL), a, start=False, stop=False)
                mm(ps_im, wsl(n, WIL), a, start=False, stop=False)

            def mm_xi(n, ps_re, ps_im, xi_h, xi_l, first):
                # the 6 terms sourced from x_im
                bsl = slice(n * tchunk, (n + 1) * tchunk)
                a, b = xi_h[:, bsl], xi_l[:, bsl]
                mm(ps_re, wsl(n, WIHN), a, start=first, stop=False)
                mm(ps_re, wsl(n, WIHN), b, start=False, stop=False)
                mm(ps_im, wsl(n, WRH), a, start=first, stop=False)
                mm(ps_im, wsl(n, WRH), b, start=False, stop=False)
                mm(ps_re, wsl(n, WILN), a, start=False, stop=True)
                mm(ps_im, wsl(n, WRL), a, start=False, stop=True)

            def evac_pair(c, j, pre0, pim0, pre1, pim1):
                # copy two blocks' psums to SBUF and store 512KB per tensor
                tsl = slice(c * tchunk, (c + 1) * tchunk)
                osb_re = opool.tile([BS, 2 * tchunk], dt.float32, name="osb_re")
                osb_im = opool.tile([BS, 2 * tchunk], dt.float32, name="osb_im")
                nc.vector.tensor_copy(osb_re[:, :tchunk], pre0)
                nc.vector.tensor_copy(osb_im[:, :tchunk], pim0)
                nc.vector.tensor_copy(osb_re[:, tchunk:], pre1)
                nc.vector.tensor_copy(osb_im[:, tchunk:], pim1)
                nc.scalar.dma_start(
                    out=ore_v[:, 2 * j : 2 * j + 2, tsl],
                    in_=osb_re.rearrange("p (n t) -> p n t", t=tchunk),
                )
                nc.scalar.dma_start(
                    out=oim_v[:, 2 * j : 2 * j + 2, tsl],
                    in_=osb_im.rearrange("p (n t) -> p n t", t=tchunk),
                )

            def psum_tiles():
                ps_re = psum_pool.tile([BS, tchunk], dt.float32, name="ps_re")
                ps_im = psum_pool.tile([BS, tchunk], dt.float32, name="ps_im")
                return ps_re, ps_im

            # ---- chunk 0: interleaved weight/x loads, xr-phase then xi-phase
            # over half-size block groups so PE starts as soon as w0+xrh land.
            load_w(0)
            xrh_t, xrl_t, xih_t, xil_t = load_x(0)
            for n in range(1, nblk):
                load_w(n)
            for half in range(2):
                blocks = range(half * 4, half * 4 + 4)
                ps = {}
                for n in blocks:
                    ps[n] = psum_tiles()
                    mm_xr(n, *ps[n], xrh_t, xrl_t, first=True)
                for n in blocks:
                    mm_xi(n, *ps[n], xih_t, xil_t, first=False)
                for j in range(half * 2, half * 2 + 2):
                    evac_pair(0, j, *ps[2 * j], *ps[2 * j + 1])

            # ---- steady chunks
            for c in range(1, nchunk):
                xrh_t, xrl_t, xih_t, xil_t = load_x(c)
                for j in range(nblk // 2):
                    pairs = []
                    for n in (2 * j, 2 * j + 1):
                        ps_re, ps_im = psum_tiles()
                        mm_xr(n, ps_re, ps_im, xrh_t, xrl_t, first=True)
                        mm_xi(n, ps_re, ps_im, xih_t, xil_t, first=False)
                        pairs += [ps_re, ps_im]
                    evac_pair(c, j, *pairs)

    nc.compile()
    return nc


def _get_module(nblk=NBLK, s=S, tchunk=TCHUNK):
    key = (nblk, s, tchunk)
    if key not in _CACHE:
        _CACHE[key] = _build_module(nblk, s, tchunk)
    return _CACHE[key]


def _split_bf16(x32):
    hi = x32.astype(BF16)
    lo = (x32 - hi.astype(np.float32)).astype(BF16)
    return np.ascontiguousarray(hi), np.ascontiguousarray(lo)


def _pack_weights(weight):
    wr = weight[..., 0].astype(np.float32)  # [n, i, o]
    wi = weight[..., 1].astype(np.float32)
    wrh = wr.astype(BF16)
    wrl = (wr - wrh.astype(np.float32)).astype(BF16)
    wih = wi.astype(BF16)
    wil = (wi - wih.astype(np.float32)).astype(BF16)
    return np.ascontiguousarray(
        np.concatenate([wrh, wrl, wih, wil, -wih, -wil], axis=2)
    )


def _setup_trace_shim():
    """Make trace=True work under axon in containers lacking antenv.axon_hooks.

    Registers a stand-in antenv.axon_hooks module whose hook drives NTFF
    capture via ctypes on libaxon_pjrt.so (mirrors trn_agent_boot), and
    disables the S3 artifact upload in bass_utils.
    """
    import contextlib
    import ctypes
    import sys
    import types

    try:
        from antenv.axon_hooks import get_axon_ntff_profile_hook  # noqa: F401

        return
    except ImportError:
        pass

    so_path = "/opt/axon/libaxon_pjrt.so"
    lib = ctypes.CDLL(so_path)
    if not hasattr(lib, "axon_start_nrt_profile"):
        return
    lib.axon_start_nrt_profile.argtypes = [
        ctypes.POINTER(ctypes.c_int64),
        ctypes.c_size_t,
    ]
    lib.axon_start_nrt_profile.restype = ctypes.c_int64
    lib.axon_stop_nrt_profile.argtypes = [ctypes.c_char_p]
    lib.axon_stop_nrt_profile.restype = ctypes.c_int64

    @contextlib.contextmanager
    def _hook(output_dir, device_ids):
        import jax

        jax.devices()
        if device_ids:
            ids = (ctypes.c_int64 * len(device_ids))(*device_ids)
            rc = lib.axon_start_nrt_profile(ids, len(device_ids))
        else:
            rc = lib.axon_start_nrt_profile(None, 0)
        if rc != 0:
            raise RuntimeError(f"axon_start_nrt_profile rc={rc}")
        try:
            yield
        finally:
            n = lib.axon_stop_nrt_profile(str(output_dir).encode())
            print(f"ntff profile: {n} file(s) written to {output_dir}")

    mod = types.ModuleType("antenv.axon_hooks")
    mod.get_axon_ntff_profile_hook = lambda: _hook
    mod.set_axon_ntff_profile_hook = lambda h: None
    sys.modules["antenv.axon_hooks"] = mod

    from concourse import bass_utils

    bass_utils.upload_artifacts = lambda tmpdir: tmpdir


def kernel(x_re, x_im, weight):
    from concourse import bass_utils

    trace = bool(int(os.environ.get("KERNEL_TRACE", "0")))
    if trace:
        _setup_trace_shim()

    nc = _get_module()
    wpack = _pack_weights(weight)

    in_maps = []
    for b in range(NCORES):
        d = {"wpack": wpack}
        d["xrh"], d["xrl"] = _split_bf16(x_re[b].T.astype(np.float32))
        d["xih"], d["xil"] = _split_bf16(x_im[b].T.astype(np.float32))
        in_maps.append(d)

    res = bass_utils.run_bass_kernel_spmd(
        nc,
        in_maps,
        core_ids=list(range(NCORES)),
        trace=trace,
    )
    kernel._last_results = res

    out = np.empty((B, S, H, 2), np.float32)
    for b in range(NCORES):
        out[b, :, :, 0] = res.results[b]["out_re"].T
        out[b, :, :, 1] = res.results[b]["out_im"].T
    return out


kernel._last_results = None
